# revision 4
# baseline (speedup 1.0000x reference)
"""Multi-head causal self-attention (B=4, S=2048, D=1024, H=16) on 8 Trainium2
NeuronCores via Bass/Tile.

Sharding: core c handles batch b = c//2 and head-half hh = c%2 (8 heads).
W_q/W_k/W_v are split column-wise (tensor parallel), W_o row-wise; each core
produces a partial (S, D) output and the host sums the two partials per batch.

Per-core device kernel (all matmuls in float32r: full-rate fp32 PE mode,
~1.5e-4 relative rms error):
  - Q^T, K^T projections in [dk, s] layout; V in [t, dk] layout with an
    appended ones-column (gives the softmax denominator for free).
  - Flash-style attention: scores computed transposed S^T[t, s] =
    (K^T)^T-chunk @ Q^T-block, exp on the scalar engine (no max subtraction:
    scores ~ N(0,1), fp32 exp cannot overflow), causal masking via an
    additive -1e30 triangle tile, unnormalized O~^T accumulated in PSUM with
    the row-sum r in row 64.
  - Normalization 1/r broadcast across partitions via a K=1 matmul with a
    ones vector, applied during the PSUM->SBUF copy.
  - Output projection accumulated over the 4 feature chunks, DMA'd out.
"""

import sys

sys.path.insert(0, "/opt/trn_rl_repo")

import numpy as np

import concourse.bacc as bacc
import concourse.mybir as mybir
from concourse.bass_utils import run_bass_kernel_spmd
from concourse.tile import TileContext

# Problem constants (hardcoded per the harness contract)
B, S, D = 4, 2048, 1024
H, DK = 16, 64
N_CORES = 8
HH = H // 2          # heads per core
F = HH * DK          # 512 features per core
P = 128
KO = D // P          # 8 contraction chunks for projections
MC = F // P          # 4 dk-feature chunks
SB = S // 512        # 4 s-blocks
TC = S // P          # 16 t-chunks
NEG = -1.0e30

F32 = mybir.dt.float32
R = mybir.dt.float32r
ADD = mybir.AluOpType.add
MULT = mybir.AluOpType.mult
EXP = mybir.ActivationFunctionType.Exp

_BUILD_CACHE: dict = {}


def _build(mode: str):
    """mode: 'causal' (tril mask) or 'full' (no masking)."""
    causal = mode == "causal"
    nc = bacc.Bacc("TRN2", target_bir_lowering=False, debug=False,
                   num_devices=N_CORES)

    xT_d = nc.dram_tensor("xT", [D, S], R, kind="ExternalInput")
    wq_d = nc.dram_tensor("wqT", [D, F], R, kind="ExternalInput")
    wk_d = nc.dram_tensor("wkT", [D, F], R, kind="ExternalInput")
    wv_d = nc.dram_tensor("wvT", [D, F], R, kind="ExternalInput")
    wo_d = nc.dram_tensor("woT", [F, D], R, kind="ExternalInput")
    tri_d = nc.dram_tensor("tri", [P, P], F32, kind="ExternalInput")
    out_d = nc.dram_tensor("out", [S, D], F32, kind="ExternalOutput")

    with TileContext(nc) as tc:
        with (
            tc.tile_pool(name="res", bufs=1) as res,
            tc.tile_pool(name="ps", bufs=1, space="PSUM") as ps,
        ):
            qT = res.tile([P, MC, S], R, name="qT")
            kT = res.tile([P, MC, S], R, name="kT")
            vA = res.tile([P, TC, HH, DK + 1], R, name="vA")
            tri_sb = res.tile([P, P], F32, name="tri_sb")
            ones_sb = res.tile([1, 64], R, name="ones_sb")
            onesP = res.tile([P, 1], F32, name="onesP")
            if causal:
                nc.sync.dma_start(tri_sb[:], tri_d[:])
            else:
                nc.vector.memset(tri_sb[:], 0.0)
            nc.vector.memset(onesP[:], 1.0)
            # memset cannot target float32r; copy-with-round from f32 ones
            nc.vector.tensor_copy(
                ones_sb[:], onesP[0:1, 0:1].to_broadcast([1, 64])
            )
            nc.vector.tensor_copy(
                vA[:, :, :, DK : DK + 1],
                onesP[:, None, None, :].to_broadcast([P, TC, HH, 1]),
            )

            # ---------------- Phase A: projections ----------------
            with tc.tile_pool(name="xw", bufs=1) as xw:
                xT_sb = xw.tile([P, KO, S], R, name="xT_sb")
                nc.sync.dma_start(
                    xT_sb[:], xT_d.rearrange("(ko p) s -> p ko s", p=P)
                )
                w_q = xw.tile([P, KO, F], R, name="w_q", tag="w", bufs=2)
                nc.sync.dma_start(w_q[:], wq_d.rearrange("(ko p) f -> p ko f", p=P))
                w_k = xw.tile([P, KO, F], R, name="w_k", tag="w", bufs=2)
                nc.sync.dma_start(w_k[:], wk_d.rearrange("(ko p) f -> p ko f", p=P))

                # Q^T, K^T: out[dk_chunk, s_block], accumulate over ko
                for w_sb, dst in ((w_q, qT), (w_k, kT)):
                    for n in range(SB):
                        for mp in range(MC // 2):
                            pt = ps.tile([P, 2, 512], F32, tag="sc", bufs=2,
                                         name="pp")
                            for half in range(2):
                                m = mp * 2 + half
                                for ko in range(KO):
                                    nc.tensor.matmul(
                                        pt[:, half],
                                        w_sb[:, ko, m * P : (m + 1) * P],
                                        xT_sb[:, ko, n * 512 : (n + 1) * 512],
                                        start=(ko == 0),
                                        stop=(ko == KO - 1),
                                    )
                            nc.vector.tensor_copy(
                                dst[:, mp * 2 : mp * 2 + 2, n * 512 : (n + 1) * 512],
                                pt[:],
                            )

                # V: out[t_chunk, dk] natural layout, scattered per head
                w_v = xw.tile([P, KO, F], R, name="w_v", tag="w", bufs=2)
                nc.sync.dma_start(w_v[:], wv_d.rearrange("(ko p) f -> p ko f", p=P))
                for scp in range(TC // 2):
                    pt = ps.tile([P, 2, 512], F32, tag="sc", bufs=2, name="pv")
                    for half in range(2):
                        sc = scp * 2 + half
                        for ko in range(KO):
                            nc.tensor.matmul(
                                pt[:, half],
                                xT_sb[:, ko, sc * P : (sc + 1) * P],
                                w_v[:, ko, :],
                                start=(ko == 0),
                                stop=(ko == KO - 1),
                            )
                    for half in range(2):
                        sc = scp * 2 + half
                        nc.vector.tensor_copy(
                            vA[:, sc, :, 0:DK],
                            pt[:, half].rearrange("p (h d) -> p h d", d=DK),
                        )

            # ---------------- Phase B: attention + output proj ----------------
            with tc.tile_pool(name="ph2", bufs=1) as ph2:
                wo_sb = ph2.tile([P, MC, D], R, name="wo_sb")
                nc.sync.dma_start(wo_sb[:], wo_d.rearrange("(fo p) n -> p fo n", p=P))

                for i in range(SB):
                    attnT = ph2.tile([P, MC, 512], R, tag="attnT", bufs=2,
                                     name="attnT")
                    for h in range(HH):
                        r0 = (h % 2) * 64
                        fc = h // 2
                        n_tc = 4 * (i + 1) if causal else TC
                        avp = ps.tile([P, 512], F32, tag="b1", bufs=3, name="avp")
                        for g in range(n_tc // 2):
                            pt = ps.tile([P, 2, 512], F32, tag="sc", bufs=2,
                                         name="sp")
                            for j in range(2):
                                c = g * 2 + j
                                nc.tensor.matmul(
                                    pt[:, j],
                                    kT[r0 : r0 + 64, fc, c * P : (c + 1) * P],
                                    qT[r0 : r0 + 64, fc, i * 512 : (i + 1) * 512],
                                    start=True,
                                    stop=True,
                                )
                            if causal:
                                for j in range(2):
                                    dlt = g * 2 + j - 4 * i
                                    if dlt >= 1:
                                        nc.vector.memset(pt[:, j, 0 : dlt * P], NEG)
                                    if dlt >= 0:
                                        sl = pt[:, j, dlt * P : (dlt + 1) * P]
                                        nc.vector.tensor_tensor(sl, sl, tri_sb[:], ADD)
                            et = ph2.tile([P, 2, 512], R, tag="e", bufs=3,
                                          name="et")
                            nc.scalar.activation(et[:], pt[:], EXP, scale=0.125)
                            for j in range(2):
                                c = g * 2 + j
                                nc.tensor.matmul(
                                    avp[0 : DK + 1, :],
                                    vA[:, c, h, :],
                                    et[:, j],
                                    start=(c == 0),
                                    stop=(c == n_tc - 1),
                                )
                        rec = ph2.tile([1, 512], R, tag="rec", bufs=2, name="rec")
                        with nc.allow_low_precision(reason="f32r softmax denom"):
                            nc.vector.reciprocal(rec[:], avp[64:65, :])
                        bc = ps.tile([64, 512], F32, tag="bc", bufs=1, name="bc")
                        nc.tensor.matmul(
                            bc[:], ones_sb[:], rec[:],
                            start=True, stop=True,
                        )
                        # walrus rejects tensor_tensor with two PSUM operands;
                        # stage the broadcast through SBUF
                        bcs = ph2.tile([64, 512], R, tag="bcs", bufs=2, name="bcs")
                        nc.vector.tensor_copy(bcs[:], bc[:])
                        nc.vector.tensor_tensor(
                            attnT[r0 : r0 + 64, fc, :], avp[0:DK, :], bcs[:], MULT
                        )

                    # output projection for this s_block
                    for scn in range(4):
                        for nh in range(2):
                            fp = ps.tile([P, 512], F32, tag="b1", bufs=3, name="fp")
                            for fcc in range(MC):
                                nc.tensor.matmul(
                                    fp[:],
                                    attnT[:, fcc, scn * P : (scn + 1) * P],
                                    wo_sb[:, fcc, nh * 512 : (nh + 1) * 512],
                                    start=(fcc == 0),
                                    stop=(fcc == MC - 1),
                                )
                            ob = ph2.tile([P, 512], F32, tag="ob", bufs=4, name="ob")
                            nc.vector.tensor_copy(ob[:], fp[:])
                            nc.sync.dma_start(
                                out_d[
                                    i * 512 + scn * P : i * 512 + (scn + 1) * P,
                                    nh * 512 : (nh + 1) * 512,
                                ],
                                ob[:],
                            )
    nc.compile()
    return nc


def get_module(mode: str):
    if mode not in _BUILD_CACHE:
        _BUILD_CACHE[mode] = _build(mode)
    return _BUILD_CACHE[mode]


def _shard_inputs(x, W_q, W_k, W_v, W_o, causal: bool):
    tri = np.where(
        np.arange(P)[:, None] <= np.arange(P)[None, :], 0.0, NEG
    ).astype(np.float32)
    in_maps = []
    for c in range(N_CORES):
        b, hh = c // 2, c % 2
        sl = slice(hh * F, (hh + 1) * F)
        in_maps.append(
            {
                "xT": np.ascontiguousarray(x[b].T),
                "wqT": np.ascontiguousarray(W_q[sl, :].T),
                "wkT": np.ascontiguousarray(W_k[sl, :].T),
                "wvT": np.ascontiguousarray(W_v[sl, :].T),
                "woT": np.ascontiguousarray(W_o[:, sl].T),
                "tri": tri,
            }
        )
    return in_maps


def _reference_fallback(x, W_q, W_k, W_v, W_o, mask):
    """Numpy fallback for arbitrary (non-causal, non-trivial) masks."""
    out = np.zeros((B, S, D), dtype=np.float32)
    m01 = np.broadcast_to(np.asarray(mask) != 0, (B, H, S, S))
    for b in range(B):
        q = (x[b] @ W_q.T).reshape(S, H, DK)
        k = (x[b] @ W_k.T).reshape(S, H, DK)
        v = (x[b] @ W_v.T).reshape(S, H, DK)
        attn = np.zeros((S, H, DK), dtype=np.float32)
        for h in range(H):
            sc = (q[:, h] @ k[:, h].T) / np.sqrt(np.float32(DK))
            sc = np.where(m01[b, h], sc, -np.inf)
            sc = sc - sc.max(axis=1, keepdims=True)
            e = np.exp(sc)
            attn[:, h] = (e @ v[:, h]) / e.sum(axis=1, keepdims=True)
        out[b] = attn.reshape(S, H * DK) @ W_o.T
    return out


def kernel(x, W_q, W_k, W_v, W_o, mask):
    x = np.ascontiguousarray(np.asarray(x), dtype=np.float32)
    W_q = np.asarray(W_q, dtype=np.float32)
    W_k = np.asarray(W_k, dtype=np.float32)
    W_v = np.asarray(W_v, dtype=np.float32)
    W_o = np.asarray(W_o, dtype=np.float32)

    m01 = np.asarray(mask)[0, 0] != 0
    if bool(m01.all()):
        mode = "full"
    elif bool((m01 == np.tril(np.ones((S, S), dtype=bool))).all()):
        mode = "causal"
    else:
        return _reference_fallback(x, W_q, W_k, W_v, W_o, mask)

    nc = get_module(mode)
    in_maps = _shard_inputs(x, W_q, W_k, W_v, W_o, mode == "causal")
    res = run_bass_kernel_spmd(nc, in_maps, core_ids=list(range(N_CORES)))
    out = np.empty((B, S, D), dtype=np.float32)
    for b in range(B):
        out[b] = res.results[2 * b]["out"] + res.results[2 * b + 1]["out"]
    return out
